# revision 1
# baseline (speedup 1.0000x reference)
"""DigitCaps1D routing kernel for 8 trn2 NeuronCores.

Strategy: shard N=8192 across the 8 cores (N_c=1024 each, full B=64).
Per-core HBM traffic is just its W/x shard (~11MB in two layouts); the
routing loop runs entirely out of SBUF, with 3 tiny (40KB) AllReduces
for the cross-core s_j sums.

Math (per routing iter r, V_r = sum_{k<r} v_k, V_0 = 0):
  u[b,n,o,d] = sum_i W[n,o,d,i] x[b,n,i]
  t_r[b,n,o] = sum_d u * V_r          (b_ij is linear in u)
  c_r        = softmax_o(t_r)
  s_r[b,o,d] = sum_n c_r * u = sum_{n,i} (c_r x)[b,n,i] W[n,o,d,i]
  v_r        = squash(s_r)
u is never materialized: t_r comes via A_o = sum_d V_r W (PE) then
sum_i x*A (DVE mult+reduce); s_r via y = c*x (DVE) then a big-K
matmul against W (PE).

n-local decomposition: n = q*8 + j*2 + h, q in [0,128), j in [0,4), h in {0,1}.
Partition layouts:
  Ws  [q, (o,i,j,h,d)] fp16  - s-matmul weights
  Wd  [(j*2+h)*16+d, (o,q,i)] f32 - A-prod rhs
  x2  [q, (i,j,h,b)] fp16    - s-matmul rhs (r0) / y source
  x3  [b*2+h, (j,q,i)] f32   - A-consume multiplier
  t,c [b*2+h, ...]           - b-major
  c_T [q, (o,j,h,b)] fp16    - after PE transpose
"""

import numpy as np
from contextlib import ExitStack

import ml_dtypes
import concourse.bass as bass
import concourse.mybir as mybir
from concourse.bass_utils import run_bass_kernel_spmd

F32 = mybir.dt.float32
F16 = mybir.dt.float16
AF = mybir.ActivationFunctionType
ALU = mybir.AluOpType
AX = mybir.AxisListType

B, N, DI, O, DO = 64, 8192, 8, 10, 16
NCORES = 8
NC = N // NCORES          # 1024 n per core
Q, J, H = 128, 4, 2       # n = q*8 + j*2 + h
OJ = O * J                # 40
R = 3


def build_nc(R=R, reps=1, fake_cc=False):
    nc = bass.Bass(detect_race_conditions=False)

    Ws = nc.declare_dram_parameter("Ws", [Q, O * DI * J * H * DO], F16, isOutput=False)
    Wd = nc.declare_dram_parameter("Wd", [Q, O * Q * DI], F32, isOutput=False)
    x2 = nc.declare_dram_parameter("x2", [Q, DI * J * H * B], F16, isOutput=False)
    x3 = nc.declare_dram_parameter("x3", [Q, J * Q * DI], F32, isOutput=False)
    ident = nc.declare_dram_parameter("ident", [Q, Q], F32, isOutput=False)
    out = nc.declare_dram_parameter("out", [DO, B * O], F32, isOutput=True)

    cc_in = [nc.dram_tensor(f"cc_in{r}", [DO, B * O], F32) for r in range(R)]
    cc_out = [
        nc.dram_tensor(f"cc_out{r}", [DO, B * O], F32, addr_space="Shared")
        for r in range(R)
    ]

    ctx = ExitStack()
    sb = ctx.enter_context

    ws_sb = sb(nc.sbuf_tensor([Q, O, DI, J, H, DO], F16))
    wd_sb = sb(nc.sbuf_tensor([Q, O, Q, DI], F32))
    x2_sb = sb(nc.sbuf_tensor([Q, DI, J, H, B], F16))
    x3_sb = sb(nc.sbuf_tensor([Q, J, Q, DI], F32))
    id_sb = sb(nc.sbuf_tensor([Q, Q], F32))
    vt_sb = sb(nc.sbuf_tensor([Q, O, B, H], F32))       # Vtilde4 A-prod weights
    prod_sb = sb(nc.sbuf_tensor([Q, 2, Q * DI], F32))   # A*x3, 2 slots
    t_sb = sb(nc.sbuf_tensor([Q, O, J, Q], F32))
    exp_sb = sb(nc.sbuf_tensor([Q, J, Q, O], F32))
    z_sb = sb(nc.sbuf_tensor([Q, J, Q], F32))
    rz_sb = sb(nc.sbuf_tensor([Q, J, Q], F32))
    c_sb = sb(nc.sbuf_tensor([Q, J, Q, O], F32))
    ct_sb = sb(nc.sbuf_tensor([Q, O, J, H, B], F16))   # c transposed
    y_sb = sb(nc.sbuf_tensor([Q, 2, DI * J * H * B], F16))  # y = c*x, 2 slots
    s_sb = sb(nc.sbuf_tensor([DO, B, O], F32))          # local s (b,o order)
    sall_sb = sb(nc.sbuf_tensor([DO, B, O], F32))       # allreduced s
    st_sb = sb(nc.sbuf_tensor([DO, B, O], F32))         # s_true (scaled r0)
    sq_sb = sb(nc.sbuf_tensor([DO, B, O], F32))
    nrm_sb = sb(nc.sbuf_tensor([DO, B], F32))
    p1_sb = sb(nc.sbuf_tensor([DO, B], F32))
    sqt_sb = sb(nc.sbuf_tensor([DO, B], F32))
    r1_sb = sb(nc.sbuf_tensor([DO, B], F32))
    p1b_sb = sb(nc.sbuf_tensor([DO, B], F32))
    r2_sb = sb(nc.sbuf_tensor([DO, B], F32))
    scl_sb = sb(nc.sbuf_tensor([DO, B], F32))
    sc2_sb = sb(nc.sbuf_tensor([DO, B], F32))
    v_sb = sb(nc.sbuf_tensor([DO, B, O], F32))
    V_sb = sb(nc.sbuf_tensor([DO, O, B], F32))          # running sum of v (o-major for vt DMA)

    ps_A = sb(nc.psum_tensor([Q, 2, 1024], F32))        # A psum, 2 slots x 2 banks
    ps_T = sb(nc.psum_tensor([Q, 2, 512], F32))         # transpose psum, 2 slots (bank-padded)
    ps_S = sb(nc.psum_tensor([DO, O, B], F32))          # s psum [16, 640]

    sem_dma = sb(nc.semaphore("sem_dma"))
    sem_cc = sb(nc.semaphore("sem_cc"))
    sem_gp = sb(nc.semaphore("sem_gp"))
    sem_pe = sb(nc.semaphore("sem_pe"))
    sem_dve = sb(nc.semaphore("sem_dve"))
    sem_act = sb(nc.semaphore("sem_act"))
    sem_ser = sb(nc.semaphore("sem_ser"))

    # ---- precomputed semaphore marks (must match emission order below) ----
    IN_DONE = 5 * 16
    d = IN_DONE
    DMA = {}
    for r in range(R):
        d += 16; DMA[("ccin", r)] = d
        d += 16; DMA[("sall", r)] = d
        if r < R - 1:
            d += 8 * 16; DMA[("vt", r)] = d
    d += 16; DMA["out"] = d

    PE_TOT = 1 + (R - 1) * 90
    DVE_TOT = 3 + (R - 1) * 94
    ACT_TOT = 2 * R - 1
    DMA_TOT = d
    CC_TOT = R

    def PE_A(r, oj, rep): return rep * PE_TOT + 1 + (r - 1) * 90 + (oj + 1)
    def PE_CT(r, oj, rep): return rep * PE_TOT + 1 + (r - 1) * 90 + 40 + (oj + 1)
    def PE_S(r, o, rep): return rep * PE_TOT + 1 + (r - 1) * 90 + 80 + (o + 1)
    def PE_S0(rep): return rep * PE_TOT + 1

    def DVE_base(r, rep): return rep * DVE_TOT + 3 + (r - 1) * 94
    def CONS(r, oj, rep): return DVE_base(r, rep) + oj + 1
    def C_READY(r, rep): return DVE_base(r, rep) + 41
    def CTD(r, oj, rep): return DVE_base(r, rep) + 41 + oj + 1
    def Y(r, o, rep): return DVE_base(r, rep) + 81 + o + 1
    def SDRAIN(r, rep): return rep * DVE_TOT + (1 if r == 0 else 3 + (r - 1) * 94 + 92)
    def P1(r, rep): return rep * DVE_TOT + (2 if r == 0 else 3 + (r - 1) * 94 + 93)
    def VUPD(r, rep): return rep * DVE_TOT + (3 if r == 0 else 3 + (r - 1) * 94 + 94)
    def ACT_SQRT(r, rep): return rep * ACT_TOT + (1 if r == 0 else 2 * r + 1)
    def ACT_EXP(r, rep): return rep * ACT_TOT + 2 * r
    def DMA_M(key, rep): return DMA[key] + rep * (DMA_TOT - IN_DONE)

    with nc.Block() as block:

        @block.sync
        def _(sync):
            sync.dma_start(out=ws_sb[:], in_=Ws[:]).then_inc(sem_dma, 16)
            sync.dma_start(out=wd_sb[:], in_=Wd[:]).then_inc(sem_dma, 16)
            sync.dma_start(out=x2_sb[:], in_=x2[:]).then_inc(sem_dma, 16)
            sync.dma_start(out=x3_sb[:], in_=x3[:]).then_inc(sem_dma, 16)
            sync.dma_start(out=id_sb[:], in_=ident[:]).then_inc(sem_dma, 16)
            for rep in range(reps):
                for r in range(R):
                    sync.wait_ge(sem_dve, SDRAIN(r, rep))
                    sync.dma_start(out=cc_in[r][:], in_=s_sb[:]).then_inc(sem_dma, 16)
                    sync.wait_ge(sem_cc, rep * CC_TOT + r + 1)
                    sync.dma_start(out=sall_sb[:], in_=cc_out[r][:]).then_inc(sem_dma, 16)
                    if r < R - 1:
                        sync.wait_ge(sem_dve, VUPD(r, rep))
                        if r == 0 and rep == 0:
                            sync.wait_ge(sem_gp, 1)  # vt_sb memset done
                        with nc.allow_non_contiguous_dma(reason="tiny V scatter (640 elems)"):
                            for j in range(J):
                                for h in range(H):
                                    # vt[32j+16h+d, (o, b, h'=h)] = V[d, (b, o)]
                                    dst = vt_sb[32 * j + 16 * h:32 * j + 16 * h + DO, :, :, h]
                                    src = V_sb[:]
                                    sync.dma_start(out=dst, in_=src).then_inc(sem_dma, 16)
                sync.wait_ge(sem_dve, VUPD(R - 1, rep))
                sync.dma_start(out=out[:], in_=v_sb.rearrange("d b o -> d (b o)")).then_inc(sem_dma, 16)

        @block.gpsimd
        def _(gpsimd):
            gpsimd.memset(vt_sb[:], 0.0).then_inc(sem_gp, 1)
            for rep in range(reps):
                for r in range(R):
                    gpsimd.wait_ge(sem_dma, DMA_M(("ccin", r), rep))
                    if fake_cc:
                        nc.gpsimd.dma_start(out=cc_out[r][:], in_=cc_in[r][:]).then_inc(sem_cc, 1)
                    else:
                        nc.gpsimd.collective_compute(
                            "AllReduce", ALU.add,
                            replica_groups=[list(range(NCORES))],
                            ins=[cc_in[r][:]], outs=[cc_out[r][:]],
                        ).then_inc(sem_cc, 1)

        @block.tensor
        def _(tensor):
            tensor.wait_ge(sem_dma, IN_DONE)
            for rep in range(reps):
                # ---- r0: s0 = sum_{n,i} W x  (c uniform folded into squash) ----
                if rep > 0:
                    tensor.wait_ge(sem_dve, SDRAIN(R - 1, rep - 1))
                for o in range(O):
                    first = True
                    for i in range(DI):
                        for j in range(J):
                            for h in range(H):
                                last = (i == DI - 1 and j == J - 1 and h == H - 1)
                                mm = nc.tensor.matmul(
                                    ps_S[:, o, :],
                                    ws_sb[:, o, i, j, h, :],
                                    x2_sb[:, i, j, h, :],
                                    start=first, stop=last,
                                )
                                first = False
                                if o == O - 1 and last:
                                    mm.then_inc(sem_pe, 1)
                for r in range(1, R):
                    tensor.wait_ge(sem_dma, DMA_M(("vt", r - 1), rep))
                    for oj in range(OJ):
                        o, j = oj // J, oj % J
                        if oj >= 2 or rep > 0:
                            w_oj, w_rep = (oj - 2, rep) if oj >= 2 else (OJ - 2 + oj, rep if r > 1 else rep - 1)
                            w_r = r if oj >= 2 else (r - 1 if r > 1 else R - 1)
                            tensor.wait_ge(sem_dve, CONS(w_r, w_oj, w_rep))
                        slot = oj % 2
                        lhsT = vt_sb[32 * j:32 * (j + 1), o, :, :]
                        for half in range(2):
                            mm = nc.tensor.matmul(
                                ps_A[:, slot, 512 * half:512 * (half + 1)],
                                lhsT,
                                wd_sb[32 * j:32 * (j + 1), o, 64 * half:64 * (half + 1), :],
                                start=True, stop=True,
                                tile_position=(32 * j, 0),
                            )
                        mm.then_inc(sem_pe, 1)
                    for oj in range(OJ):
                        o, j = oj // J, oj % J
                        if oj >= 2:
                            tensor.wait_ge(sem_dve, CTD(r, oj - 2, rep))
                        else:
                            tensor.wait_ge(sem_dve, C_READY(r, rep))
                        slot = oj % 2
                        nc.tensor.matmul(
                            ps_T[:, slot, 0:Q],
                            c_sb[:, j, :, o],
                            id_sb[:],
                            start=True, stop=True,
                        ).then_inc(sem_pe, 1)
                    for o in range(O):
                        tensor.wait_ge(sem_dve, Y(r, o, rep))
                        slot = o % 2
                        first = True
                        for i in range(DI):
                            for j in range(J):
                                for h in range(H):
                                    last = (i == DI - 1 and j == J - 1 and h == H - 1)
                                    mm = nc.tensor.matmul(
                                        ps_S[:, o, :],
                                        ws_sb[:, o, i, j, h, :],
                                        y_sb[:, slot, :].rearrange(
                                            "q (i j h b) -> q i j h b", i=DI, j=J, h=H
                                        )[:, i, j, h, :],
                                        start=first, stop=last,
                                    )
                                    first = False
                        mm.then_inc(sem_pe, 1)

        @block.vector
        def _(vector):
            class _Ser: n = 0
            ser = _Ser()
            def squash(r, rep):
                # sall [16,(b,o)]; r0 scales by 1/O
                vector.wait_ge(sem_dma, DMA_M(("sall", r), rep))
                if r == 0:
                    nc.vector.tensor_scalar_mul(st_sb[:], sall_sb[:], 0.1)
                else:
                    nc.vector.tensor_copy(st_sb[:], sall_sb[:])
                nc.vector.tensor_mul(sq_sb[:], st_sb[:], st_sb[:])
                nc.vector.tensor_reduce(nrm_sb[:], sq_sb[:], axis=AX.X, op=ALU.add)
                nc.vector.tensor_scalar_add(p1_sb[:], nrm_sb[:], 1e-9).then_inc(sem_dve, 1)  # P1
                vector.wait_ge(sem_act, ACT_SQRT(r, rep))
                nc.vector.tensor_scalar_add(p1b_sb[:], nrm_sb[:], 1.0).then_inc(sem_ser, 1)
                ser.n += 1; vector.wait_ge(sem_ser, ser.n)
                nc.vector.reciprocal(r2_sb[:], sqt_sb[:])
                nc.vector.reciprocal(r1_sb[:], p1b_sb[:]).then_inc(sem_ser, 1)
                ser.n += 1; vector.wait_ge(sem_ser, ser.n)
                nc.vector.tensor_mul(scl_sb[:], nrm_sb[:], r1_sb[:])
                nc.vector.tensor_mul(sc2_sb[:], scl_sb[:], r2_sb[:])
                # v = scale (bcast over o) * s_true
                scl_ap = sc2_sb[:]
                scl_b = bass.AP(
                    tensor=scl_ap.tensor, offset=scl_ap.offset,
                    ap=list(scl_ap.ap) + [[0, O]],
                )
                nc.vector.tensor_mul(v_sb[:], st_sb[:], scl_b)
                if r == 0:
                    nc.vector.tensor_copy(V_sb[:], v_sb.rearrange("d b o -> d o b")).then_inc(sem_dve, 1)  # VUPD
                else:
                    nc.vector.tensor_add(V_sb[:], V_sb[:], v_sb.rearrange("d b o -> d o b")).then_inc(sem_dve, 1)

            for rep in range(reps):
                # ---- r0 ----
                vector.wait_ge(sem_pe, PE_S0(rep))
                nc.vector.tensor_copy(
                    s_sb[:], ps_S.rearrange("d o b -> d b o")
                ).then_inc(sem_dve, 1)  # SDRAIN(0)
                squash(0, rep)
                for r in range(1, R):
                    for oj in range(OJ):
                        o, j = oj // J, oj % J
                        slot = oj % 2
                        vector.wait_ge(sem_pe, PE_A(r, oj, rep))
                        nc.vector.tensor_mul(
                            prod_sb[:, slot, :], ps_A[:, slot, 0:1024], x3_sb[:, j, :, :]
                        )
                        nc.vector.tensor_reduce(
                            t_sb[:, o, j, :],
                            prod_sb[:, slot, :].rearrange("p (q i) -> p q i", i=DI),
                            axis=AX.X, op=ALU.add,
                        ).then_inc(sem_dve, 1)  # CONS
                    vector.wait_ge(sem_act, ACT_EXP(r, rep))
                    nc.vector.tensor_reduce(z_sb[:], exp_sb[:], axis=AX.X, op=ALU.add).then_inc(sem_ser, 1)
                    ser.n += 1; vector.wait_ge(sem_ser, ser.n)
                    nc.vector.reciprocal(rz_sb[:], z_sb[:]).then_inc(sem_ser, 1)
                    ser.n += 1; vector.wait_ge(sem_ser, ser.n)
                    rz_ap = rz_sb[:]
                    rz_b = bass.AP(
                        tensor=rz_ap.tensor, offset=rz_ap.offset,
                        ap=list(rz_ap.ap) + [[0, O]],
                    )
                    nc.vector.tensor_mul(c_sb[:], exp_sb[:], rz_b).then_inc(sem_dve, 1)  # C_READY
                    for oj in range(OJ):
                        o, j = oj // J, oj % J
                        slot = oj % 2
                        vector.wait_ge(sem_pe, PE_CT(r, oj, rep))
                        nc.vector.tensor_copy(
                            ct_sb[:, o, j, :, :],
                            ps_T[:, slot, 0:Q].rearrange("q (b h) -> q h b", h=H),
                        ).then_inc(sem_dve, 1)  # CTD
                    for o in range(O):
                        slot = o % 2
                        if o >= 2:
                            vector.wait_ge(sem_pe, PE_S(r, o - 2, rep))
                        ct_o = ct_sb[:, o, :, :, :]
                        ct_b = bass.AP(
                            tensor=ct_o.tensor, offset=ct_o.offset,
                            ap=[ct_o.ap[0], [0, DI], ct_o.ap[1], ct_o.ap[2], ct_o.ap[3]],
                        )
                        nc.vector.tensor_mul(
                            y_sb[:, slot, :],
                            x2_sb.rearrange("q i j h b -> q (i j h b)"),
                            ct_b,
                        ).then_inc(sem_dve, 1)  # Y
                    vector.wait_ge(sem_pe, PE_S(r, O - 1, rep))
                    nc.vector.tensor_copy(
                        s_sb[:], ps_S.rearrange("d o b -> d b o")
                    ).then_inc(sem_dve, 1)  # SDRAIN
                    squash(r, rep)

        @block.scalar
        def _(scalar):
            def do_sqrt():
                nc.scalar.activation(sqt_sb[:], p1_sb[:], AF.Sqrt).then_inc(sem_act, 1)
            for rep in range(reps):
                scalar.wait_ge(sem_dve, P1(0, rep))
                do_sqrt()
                for r in range(1, R):
                    scalar.wait_ge(sem_dve, CONS(r, OJ - 1, rep))
                    nc.scalar.activation(
                        exp_sb[:],
                        t_sb.rearrange("p o j q -> p j q o"),
                        AF.Exp,
                    ).then_inc(sem_act, 1)
                    scalar.wait_ge(sem_dve, P1(r, rep))
                    do_sqrt()

    return nc, ctx


def _prep_core(x_c, W_c):
    bf = np.float16
    xr = np.ascontiguousarray(x_c).reshape(B, Q, J, H, DI)        # b q j h i
    x2 = xr.transpose(1, 4, 2, 3, 0).reshape(Q, DI * J * H * B)
    x3 = xr.transpose(0, 3, 2, 1, 4).reshape(Q, J * Q * DI)       # (b h) j q i
    wr = np.ascontiguousarray(W_c).reshape(Q, J, H, O, DO, DI)    # q j h o d i
    ws = wr.transpose(0, 3, 5, 1, 2, 4).reshape(Q, O * DI * J * H * DO)
    wd = wr.transpose(1, 2, 4, 3, 0, 5).reshape(Q, O * Q * DI)    # (j h d) o q i
    return {
        "Ws": np.ascontiguousarray(ws).astype(bf),
        "Wd": np.ascontiguousarray(wd).astype(np.float32),
        "x2": np.ascontiguousarray(x2).astype(bf),
        "x3": np.ascontiguousarray(x3).astype(np.float32),
        "ident": np.eye(Q, dtype=np.float32),
    }


def kernel(x, W):
    x = np.asarray(x, dtype=np.float32)
    W = np.asarray(W, dtype=np.float32)
    nc, ctx = build_nc()
    in_maps = [
        _prep_core(x[:, c * NC:(c + 1) * NC, :], W[0, c * NC:(c + 1) * NC])
        for c in range(NCORES)
    ]
    res = run_bass_kernel_spmd(nc, in_maps, list(range(NCORES)))
    ctx.close()
    o = res.results[0]["out"]                      # [16, 640]
    return np.ascontiguousarray(
        o.reshape(DO, B, O).transpose(1, 2, 0)
    ).astype(np.float32)


if __name__ == "__main__":
    rng = np.random.default_rng(0)
    x = rng.standard_normal((B, N, DI), dtype=np.float32)
    W = (0.01 * rng.standard_normal((1, N, O, DO, DI))).astype(np.float32)
    v = kernel(x, W)
    print("out shape:", v.shape, "std:", v.std())

